# revision 21
# baseline (speedup 1.0000x reference)
"""Trainium2 Bass kernel for a single-layer transformer encoder.

Model: B=2, N=2048, D=1024, H=16, DFF=4096 (pre-computed QKV attention +
residual/LN + GELU FFN + residual/LN).

Sharding (zero-collective): 2 batches x 4-way sequence split. Core c owns
the 512 query tokens q=c%4 of batch b=c//4 and recomputes K/V for its whole
batch locally (~1.37x compute redundancy, but no collectives at all).

Device layout is feature-major ("transposed"): activations are stored as
[feature, token] so every projection's weight matrix is the natural
stationary (lhsT) operand and activations stream as the moving operand.
Softmax runs on transposed scores PT[j, i] = exp(scale * k_j . q_i); the
denominators come for free from a ones-column appended to V (out partition
64 of the attention-output accumulation), so no cross-partition reduction
is ever needed. LayerNorm reductions over the feature (partition) dim are
done with ones-vector matmuls on the PE; per-token mean/rstd are broadcast
back across partitions with rank-1 (k=1) matmuls.

Precision: matmul operands are bf16 (input rounding washes out through the
LayerNorms; measured ~7e-4 rel err end to end), while the residual spine
(x_own, z1, xln1 residual copy, z2) and all LN statistics stay fp32.
Attention exp runs on the Scalar engine over [128, 1024] double-bank PSUM
tiles (halves the per-instruction overhead); everything else that would
touch ACT's table switcher (copies, scale/bias applies) runs on Vector.
"""

import os
import sys

for _p in ("/opt/trn_rl_repo", "/root/.axon_site", "/root/.axon_site/_ro/trn_rl_repo"):
    if os.path.isdir(_p) and _p not in sys.path:
        sys.path.append(_p)

import numpy as np

import concourse.bacc as bacc
import concourse.mybir as mybir
import concourse.tile as tile
from concourse.bass_utils import run_bass_kernel_spmd

P = 128
B, NSEQ, D, H, DFF = 2, 2048, 1024, 16, 4096
DH = D // H                     # 64
NT = 512                        # query tokens per core
DM = D // P                     # 8 feature chunks
JC = NSEQ // P                  # 16 key-token chunks
TC = NSEQ // 512                # 4 512-token chunks
FC = DFF // P                   # 32 FFN feature chunks
HPAIRS = H // 2                 # 8
SCALE = DH ** -0.5
EPS = 1e-5

F32 = mybir.dt.float32
BF16 = mybir.dt.bfloat16
AF = mybir.ActivationFunctionType

_NC_CACHE = None


def _rearr(ap):
    """DRAM [D_like, T] -> [p, chunk, T] view with chunk-major features."""
    return ap.rearrange("(c p) t -> p c t", p=P)


def _build_nc(reps=1, phases=("qkv", "attn", "proj", "ffn")):
    nc = bacc.Bacc("TRN2", target_bir_lowering=False, debug=False)

    xT = nc.dram_tensor("xT", [D, NSEQ], BF16, kind="ExternalInput")
    x_own = nc.dram_tensor("x_own", [D, NT], F32, kind="ExternalInput")
    w_qkv = nc.dram_tensor("w_qkv", [D, 3 * D], BF16, kind="ExternalInput")
    w_out = nc.dram_tensor("w_out", [D, D], BF16, kind="ExternalInput")
    w1 = nc.dram_tensor("w1", [D, DFF], BF16, kind="ExternalInput")
    w2 = nc.dram_tensor("w2", [DFF, D], BF16, kind="ExternalInput")
    b1 = nc.dram_tensor("b1", [DFF], F32, kind="ExternalInput")
    b2 = nc.dram_tensor("b2", [D], F32, kind="ExternalInput")
    ln1_w = nc.dram_tensor("ln1_w", [D], F32, kind="ExternalInput")
    ln1_b = nc.dram_tensor("ln1_b", [D], F32, kind="ExternalInput")
    ln2_w = nc.dram_tensor("ln2_w", [D], F32, kind="ExternalInput")
    ln2_b = nc.dram_tensor("ln2_b", [D], F32, kind="ExternalInput")
    yT = nc.dram_tensor("yT", [D, NT], F32, kind="ExternalOutput")

    with tile.TileContext(nc) as tc, \
         nc.allow_low_precision(reason="bf16 matmul operands; fp32 spine"):
        for _ in range(reps):
            _emit(nc, tc, xT, x_own, w_qkv, w_out, w1, w2, b1, b2,
                  ln1_w, ln1_b, ln2_w, ln2_b, yT, phases=phases)
    nc.compile()
    return nc


def _emit(nc, tc, xT_d, xown_d, w_qkv, w_out, w1, w2, b1, b2,
          ln1_w, ln1_b, ln2_w, ln2_b, yT_d,
          phases=("qkv", "attn", "proj", "ffn")):
    # ---------------- whole-kernel pools ----------------
    with tc.tile_pool(name="const", bufs=1) as pc, \
         tc.tile_pool(name="pers", bufs=1) as pers, \
         tc.tile_pool(name="scratch", bufs=3) as sq_pool, \
         tc.tile_pool(name="vecs", bufs=6) as vec_pool, \
         tc.tile_pool(name="psacc", bufs=2, space="PSUM") as psacc, \
         tc.tile_pool(name="pspt", bufs=2, space="PSUM") as pspt, \
         tc.tile_pool(name="psout", bufs=2, space="PSUM") as psout:

        # ---------------- constants ----------------
        ones_f32 = pc.tile([P, 2 * P], F32)
        nc.vector.memset(ones_f32[:], 1.0)
        ones_col = pc.tile([P, 1], BF16)          # lhsT for partition-sums
        nc.vector.tensor_copy(ones_col[:], ones_f32[:, 0:1])
        ones_row = pc.tile([1, P], F32)           # lhsT for exact broadcasts
        nc.vector.tensor_copy(ones_row[:], ones_f32[0:1, 0:P])
        eps_sb = pc.tile([1, 1], F32)
        nc.vector.memset(eps_sb[:], EPS)
        b1_sb = pc.tile([P, FC], F32)
        nc.sync.dma_start(b1_sb[:], b1.ap().rearrange("(c p) -> p c", p=P))
        b2_sb = pc.tile([P, DM], F32)
        nc.sync.dma_start(b2_sb[:], b2.ap().rearrange("(c p) -> p c", p=P))
        lnw1_sb = pc.tile([P, DM], F32)
        nc.sync.dma_start(lnw1_sb[:], ln1_w.ap().rearrange("(c p) -> p c", p=P))
        lnb1_sb = pc.tile([P, DM], F32)
        nc.sync.dma_start(lnb1_sb[:], ln1_b.ap().rearrange("(c p) -> p c", p=P))
        lnw2_sb = pc.tile([P, DM], F32)
        nc.sync.dma_start(lnw2_sb[:], ln2_w.ap().rearrange("(c p) -> p c", p=P))
        lnb2_sb = pc.tile([P, DM], F32)
        nc.sync.dma_start(lnb2_sb[:], ln2_b.ap().rearrange("(c p) -> p c", p=P))

        # persistent activations; same-tag pairs share one slot (the second
        # tile's writes wait on the first's last reads via pool rotation)
        QT = pers.tile([P, DM, NT], BF16, tag="ta",
                       padded_shape=[P, DM, 2 * NT])
        outT = pers.tile([P, DM, NT], BF16, tag="tb",
                         padded_shape=[P, DM, 2 * NT])
        xow = pers.tile([P, DM, NT], F32, tag="tc")  # own-token x (residual 1)
        xln1 = pers.tile([P, DM, NT], BF16)     # LN1 out (ffn matmul operand)

        nc.sync.dma_start(xow[:], _rearr(xown_d.ap()))

        def ln_apply(z_tile, writes):
            """LayerNorm over features of z_tile [P, DM, NT] (fp32).
            writes(k, src_f32_ap) stores chunk k."""
            # bf16 shadow for the PE stat reductions (errors average out)
            s1 = psacc.tile([1, NT], F32, tag="acc")
            s2 = psacc.tile([1, NT], F32, tag="acc")
            for k in range(DM):
                zb = sq_pool.tile([P, NT], BF16, tag="sq")
                nc.vector.tensor_copy(zb[:], z_tile[:, k, :])
                nc.tensor.matmul(s1[:], ones_col[:], zb[:],
                                 start=(k == 0), stop=(k == DM - 1))
                sq = sq_pool.tile([P, NT], BF16, tag="sq")
                nc.vector.tensor_mul(sq[:], zb[:], zb[:])
                nc.tensor.matmul(s2[:], ones_col[:], sq[:],
                                 start=(k == 0), stop=(k == DM - 1))
            mu = vec_pool.tile([1, NT], F32, tag="v")
            nc.vector.tensor_scalar_mul(mu[:], s1[:], 1.0 / D)
            var = vec_pool.tile([1, NT], F32, tag="v")
            nc.vector.tensor_scalar_mul(var[:], s2[:], 1.0 / D)
            musq = vec_pool.tile([1, NT], F32, tag="v")
            nc.vector.tensor_mul(musq[:], mu[:], mu[:])
            nc.vector.tensor_sub(var[:], var[:], musq[:])
            nc.scalar.activation(var[:], var[:], AF.Sqrt, bias=eps_sb[:])
            rec = vec_pool.tile([1, NT], F32, tag="v")
            nc.vector.reciprocal(rec[:], var[:])
            murf = vec_pool.tile([1, NT], F32, tag="v")
            nc.vector.tensor_mul(murf[:], mu[:], rec[:])
            R = psacc.tile([P, NT], F32, tag="acc")
            nc.tensor.matmul(R[:], ones_row[:], rec[:], start=True, stop=True)
            MR = psacc.tile([P, NT], F32, tag="acc")
            nc.tensor.matmul(MR[:], ones_row[:], murf[:], start=True, stop=True)
            for k in range(DM):
                t = sq_pool.tile([P, NT], F32, tag="sq")
                nc.vector.tensor_mul(t[:], z_tile[:, k, :], R[:])
                nc.vector.tensor_sub(t[:], t[:], MR[:])
                writes(k, t)

        with tc.tile_pool(name="xpool", bufs=1) as px, \
             tc.tile_pool(name="ktp", bufs=1) as kt_pool, \
             tc.tile_pool(name="vpp", bufs=1) as vp_pool:
            xT = px.tile([P, DM, NSEQ], BF16)
            xTs = _rearr(xT_d.ap())
            for k in range(DM):
                nc.sync.dma_start(xT[:, k, :], xTs[:, k, :])

            # -------- projections: Q, V, K (dense PE block) --------------
            with tc.tile_pool(name="wq", bufs=2) as wq_pool:
                for qf in range(DM):
                    wq = wq_pool.tile([P, DM, P], BF16)
                    nc.sync.dma_start(
                        wq[:], _rearr(w_qkv.ap()[:, qf * P:(qf + 1) * P]))
                    acc = psacc.tile([P, NT], F32, tag="acc")
                    for k in range(DM):
                        nc.tensor.matmul(acc[:], wq[:, k, :], xT[:, k, 0:NT],
                                         start=(k == 0), stop=(k == DM - 1))
                    nc.vector.tensor_copy(QT[:, qf, :], acc[:])

            vp = vp_pool.tile([P, JC, H * 65], BF16)
            vp_h = vp.rearrange("p j (h e) -> p j h e", e=65)
            nc.vector.tensor_copy(
                vp_h[:, :, :, 64:65],
                ones_f32.rearrange("p (a b c) -> p a b c", b=H, c=1))
            with tc.tile_pool(name="wv", bufs=2) as wv_pool:
                for dvc in range(2):      # 512 v-features = 8 heads at a time
                    wv = wv_pool.tile([P, DM, 512], BF16)
                    nc.sync.dma_start(
                        wv[:], _rearr(w_qkv.ap()[:, 2 * D + dvc * 512:
                                                 2 * D + (dvc + 1) * 512]))
                    for jc in range(JC):
                        acc = psacc.tile([P, 512], F32, tag="acc")
                        for k in range(DM):
                            nc.tensor.matmul(
                                acc[:], xT[:, k, jc * P:(jc + 1) * P],
                                wv[:, k, :],
                                start=(k == 0), stop=(k == DM - 1))
                        nc.vector.tensor_copy(
                            vp_h[:, jc, dvc * 8:(dvc + 1) * 8, 0:64],
                            acc[:].rearrange("p (h e) -> p h e", e=64))

            kt = kt_pool.tile([P, DM, NSEQ], BF16)
            with tc.tile_pool(name="wk", bufs=2) as wk_pool:
                for kf in range(DM):
                    wk = wk_pool.tile([P, DM, P], BF16)
                    nc.sync.dma_start(
                        wk[:], _rearr(w_qkv.ap()[:, D + kf * P:D + (kf + 1) * P]))
                    for t in range(TC):
                        acc = psacc.tile([P, 512], F32, tag="acc")
                        for k in range(DM):
                            nc.tensor.matmul(
                                acc[:], wk[:, k, :],
                                xT[:, k, t * 512:(t + 1) * 512],
                                start=(k == 0), stop=(k == DM - 1))
                        nc.vector.tensor_copy(kt[:, kf, t * 512:(t + 1) * 512],
                                              acc[:])

            # -------- attention (8 head-pairs) ---------------------------
            if "attn" in phases:
                with tc.tile_pool(name="pt", bufs=4) as pt_pool:
                    for hp in range(HPAIRS):
                        oacc = [psout.tile([65, NT], F32, tag="o",
                                           name=f"oacc{i}") for i in range(2)]
                        for jc in range(JC):
                            pt_ps = pspt.tile([P, 2 * NT], F32, tag="pt")
                            for i in range(2):
                                rows = slice(64 * i, 64 * i + 64)
                                nc.tensor.matmul(
                                    pt_ps[:, i * NT:(i + 1) * NT],
                                    kt[rows, hp, jc * P:(jc + 1) * P],
                                    QT[rows, hp, :],
                                    start=True, stop=True)
                            pt_sb = pt_pool.tile([P, 2 * NT], BF16, tag="ptsb")
                            nc.scalar.activation(pt_sb[:], pt_ps[:], AF.Exp,
                                                 scale=SCALE)
                            for i in range(2):
                                h = 2 * hp + i
                                nc.tensor.matmul(
                                    oacc[i][:],
                                    vp[:, jc, h * 65:(h + 1) * 65],
                                    pt_sb[:, i * NT:(i + 1) * NT],
                                    start=(jc == 0), stop=(jc == JC - 1))
                        for i in range(2):
                            rec = vec_pool.tile([1, NT], F32, tag="v")
                            nc.vector.reciprocal(rec[:], oacc[i][64:65, :])
                            bc = pspt.tile([64, NT], F32, tag="pt")
                            nc.tensor.matmul(bc[:], ones_row[:, 0:64], rec[:],
                                             start=True, stop=True)
                            bc_sb = sq_pool.tile([P, NT], F32, tag="sq")
                            nc.vector.tensor_copy(bc_sb[0:64, :], bc[:])
                            nc.vector.tensor_mul(
                                outT[64 * i:64 * i + 64, hp, :],
                                oacc[i][0:64, :], bc_sb[0:64, :])

        # -------- output projection + residual 1 -------------------------
        z1 = pers.tile([P, DM, NT], F32, tag="ta")   # reuses QT's slot
        with tc.tile_pool(name="wo", bufs=2) as wo_pool:
            for ef in range(DM):
                wo = wo_pool.tile([P, DM, P], BF16)
                nc.sync.dma_start(
                    wo[:], _rearr(w_out.ap()[:, ef * P:(ef + 1) * P]))
                acc = psacc.tile([P, NT], F32, tag="acc")
                for k in range(DM):
                    nc.tensor.matmul(acc[:], wo[:, k, :], outT[:, k, :],
                                     start=(k == 0), stop=(k == DM - 1))
                nc.vector.tensor_add(z1[:, ef, :], acc[:], xow[:, ef, :])

        # -------- LN1 ----------------------------------------------------
        xln1f = pers.tile([P, DM, NT], F32, tag="tb")  # reuses outT's slot

        def write_xln1(k, t):
            nc.vector.tensor_scalar(xln1f[:, k, :], t[:],
                                    lnw1_sb[:, k:k + 1], lnb1_sb[:, k:k + 1],
                                    op0=mybir.AluOpType.mult,
                                    op1=mybir.AluOpType.add)
            nc.vector.tensor_copy(xln1[:, k, :], xln1f[:, k, :])
        ln_apply(z1, write_xln1)

        # -------- FFN ----------------------------------------------------
        if "ffn" in phases:
            with tc.tile_pool(name="hpool", bufs=1) as ph:
                hT = ph.tile([P, FC, NT], BF16)
                with tc.tile_pool(name="w1p", bufs=3) as w1_pool:
                    for fg in range(DFF // 512):
                        w1t = w1_pool.tile([P, DM, 512], BF16)
                        w1s = _rearr(w1.ap()[:, fg * 512:(fg + 1) * 512])
                        for k in range(DM):
                            nc.sync.dma_start(w1t[:, k, :], w1s[:, k, :])
                        for f4 in range(4):
                            f = fg * 4 + f4
                            acc = psacc.tile([P, NT], F32, tag="acc")
                            for k in range(DM):
                                nc.tensor.matmul(
                                    acc[:], w1t[:, k, f4 * P:(f4 + 1) * P],
                                    xln1[:, k, :],
                                    start=(k == 0), stop=(k == DM - 1))
                            nc.scalar.activation(hT[:, f, :], acc[:], AF.Gelu,
                                                 bias=b1_sb[:, f:f + 1])

                with tc.tile_pool(name="w2p", bufs=3) as w2_pool:
                    z2 = pers.tile([P, DM, NT], F32, tag="tc")  # xow's slot
                    for ef in range(DM):
                        w2t = w2_pool.tile([P, FC, P], BF16)
                        w2s = (w2.ap()[:, ef * P:(ef + 1) * P]
                               .rearrange("(c p) e -> p c e", p=P))
                        for k4 in range(8):
                            nc.sync.dma_start(w2t[:, k4 * 4:(k4 + 1) * 4, :],
                                              w2s[:, k4 * 4:(k4 + 1) * 4, :])
                        acc = psacc.tile([P, NT], F32, tag="acc")
                        for k in range(FC):
                            nc.tensor.matmul(acc[:], w2t[:, k, :], hT[:, k, :],
                                             start=(k == 0), stop=(k == FC - 1))
                        t = sq_pool.tile([P, NT], F32, tag="sq")
                        nc.vector.tensor_scalar_add(t[:], acc[:],
                                                    b2_sb[:, ef:ef + 1])
                        nc.vector.tensor_add(z2[:, ef, :], t[:],
                                             xln1f[:, ef, :])

        # -------- LN2 -> output ------------------------------------------
        with tc.tile_pool(name="outstage", bufs=2) as out_pool:
            yT_r = _rearr(yT_d.ap())

            def write_out(k, t):
                o = out_pool.tile([P, NT], F32)
                nc.vector.tensor_scalar(o[:], t[:],
                                        lnw2_sb[:, k:k + 1],
                                        lnb2_sb[:, k:k + 1],
                                        op0=mybir.AluOpType.mult,
                                        op1=mybir.AluOpType.add)
                nc.sync.dma_start(yT_r[:, k, :], o[:])
            ln_apply(z2 if "ffn" in phases else z1, write_out)  # noqa: F821


def _get_nc():
    global _NC_CACHE
    if _NC_CACHE is None:
        _NC_CACHE = _build_nc()
    return _NC_CACHE


def make_in_maps(x, w_qkv, w_out, ln1_w, ln1_b, w1, b1, w2, b2,
                 ln2_w, ln2_b):
    import ml_dtypes
    bf = ml_dtypes.bfloat16
    x = np.ascontiguousarray(np.asarray(x, dtype=np.float32))
    shared = {
        "w_qkv": np.ascontiguousarray(np.asarray(w_qkv, np.float32).astype(bf)),
        "w_out": np.ascontiguousarray(np.asarray(w_out, np.float32).astype(bf)),
        "w1": np.ascontiguousarray(np.asarray(w1, np.float32).astype(bf)),
        "w2": np.ascontiguousarray(np.asarray(w2, np.float32).astype(bf)),
        "b1": np.asarray(b1, np.float32),
        "b2": np.asarray(b2, np.float32),
        "ln1_w": np.asarray(ln1_w, np.float32),
        "ln1_b": np.asarray(ln1_b, np.float32),
        "ln2_w": np.asarray(ln2_w, np.float32),
        "ln2_b": np.asarray(ln2_b, np.float32),
    }
    in_maps = []
    for c in range(8):
        b, q = divmod(c, 4)
        xT = np.ascontiguousarray(x[b].T)             # [D, NSEQ]
        # rotate so this core's own tokens are always columns [0, NT)
        xTr = np.ascontiguousarray(np.roll(xT, -q * NT, axis=1))
        in_maps.append({
            "xT": np.ascontiguousarray(xTr.astype(bf)),
            "x_own": np.ascontiguousarray(xTr[:, 0:NT]),
            **shared,
        })
    return in_maps


def kernel(x, w_qkv, w_out, ln1_w, ln1_b, w1, b1, w2, b2, ln2_w, ln2_b):
    in_maps = make_in_maps(x, w_qkv, w_out, ln1_w, ln1_b, w1, b1, w2, b2,
                           ln2_w, ln2_b)
    nc = _get_nc()
    res = run_bass_kernel_spmd(nc, in_maps, list(range(8)))

    out = np.empty((B, NSEQ, D), np.float32)
    for c in range(8):
        b, q = divmod(c, 4)
        out[b, q * NT:(q + 1) * NT, :] = res.results[c]["yT"].T
    return out


# revision 26
# speedup vs baseline: 12.2905x; 12.2905x over previous
"""Trainium2 Bass kernel for a single-layer transformer encoder.

Model: B=2, N=2048, D=1024, H=16, DFF=4096 (pre-computed QKV attention +
residual/LN + GELU FFN + residual/LN).

Sharding (zero-collective): 2 batches x 4-way sequence split. Core c owns
the 512 query tokens q=c%4 of batch b=c//4 and recomputes K/V for its whole
batch locally (~1.37x compute redundancy, but no collectives at all).

Device layout is feature-major ("transposed"): activations are stored as
[feature, token] so every projection's weight matrix is the natural
stationary (lhsT) operand and activations stream as the moving operand.
Softmax runs on transposed scores PT[j, i] = exp(scale * k_j . q_i); the
denominators come for free from a ones-column appended to V (out partition
64 of the attention-output accumulation), so no cross-partition reduction
is ever needed. LayerNorm reductions over the feature (partition) dim are
done with ones-vector matmuls on the PE; per-token mean/rstd are broadcast
back across partitions with rank-1 (k=1) fp32 matmuls (exact).

Precision: matmul operands are bf16 (input rounding washes out through the
LayerNorms; ~7e-4 rel err end to end), while the residual spine (x_own,
z1, xln1 residual copy, z2) and all LN statistics stay fp32.

Performance structure: per-core DMA sustains only ~116 GB/s, so the 16 MB
of FFN weights are streamed on the gpsimd (SWDGE) queues starting at
attention time — the attention phase is the only compute window with no
DMA of its own. Attention exp runs on ScalarE over [128, 1024] double-bank
PSUM tiles (amortizes the +352-cycle ACTIVATE overhead); all copies and
scale/bias applies run on VectorE so ACT never switches LUT tables.
SBUF slots are aliased across phases via same-tag pool rotation
(QT/kt/vp/xow slots are reused for z1/xln1f/z2 once their readers finish).
"""

import os
import sys

for _p in ("/opt/trn_rl_repo", "/root/.axon_site", "/root/.axon_site/_ro/trn_rl_repo"):
    if os.path.isdir(_p) and _p not in sys.path:
        sys.path.append(_p)

import numpy as np

import concourse.bacc as bacc
import concourse.mybir as mybir
import concourse.tile as tile
from concourse.bass_utils import run_bass_kernel_spmd

P = 128
B, NSEQ, D, H, DFF = 2, 2048, 1024, 16, 4096
DH = D // H                     # 64
NT = 512                        # query tokens per core
DM = D // P                     # 8 feature chunks
JC = NSEQ // P                  # 16 key-token chunks
TC = NSEQ // 512                # 4 512-token chunks
FC = DFF // P                   # 32 FFN feature chunks
HPAIRS = H // 2                 # 8
SCALE = DH ** -0.5
EPS = 1e-5

F32 = mybir.dt.float32
BF16 = mybir.dt.bfloat16
AF = mybir.ActivationFunctionType

_NC_CACHE = None


def _rearr(ap):
    """DRAM [D_like, T] -> [p, chunk, T] view with chunk-major features."""
    return ap.rearrange("(c p) t -> p c t", p=P)


def _build_nc(reps=1, phases=("qkv", "attn", "proj", "ffn")):
    nc = bacc.Bacc("TRN2", target_bir_lowering=False, debug=False)

    xT = nc.dram_tensor("xT", [D, NSEQ], BF16, kind="ExternalInput")
    x_own = nc.dram_tensor("x_own", [D, NT], F32, kind="ExternalInput")
    # weights arrive pre-tiled: [out_chunk, partition, in_chunk, out_cols]
    w_q = nc.dram_tensor("w_q", [DM, P, DM, P], BF16, kind="ExternalInput")
    w_k = nc.dram_tensor("w_k", [DM, P, DM, P], BF16, kind="ExternalInput")
    w_v = nc.dram_tensor("w_v", [2, P, DM, 512], BF16, kind="ExternalInput")
    w_out = nc.dram_tensor("w_out", [DM, P, DM, P], BF16, kind="ExternalInput")
    w1 = nc.dram_tensor("w1", [DFF // 512, P, DM, 512], BF16,
                        kind="ExternalInput")
    w2 = nc.dram_tensor("w2", [DM, P, FC, P], BF16, kind="ExternalInput")
    b1 = nc.dram_tensor("b1", [DFF], F32, kind="ExternalInput")
    b2 = nc.dram_tensor("b2", [D], F32, kind="ExternalInput")
    ln1_w = nc.dram_tensor("ln1_w", [D], F32, kind="ExternalInput")
    ln1_b = nc.dram_tensor("ln1_b", [D], F32, kind="ExternalInput")
    ln2_w = nc.dram_tensor("ln2_w", [D], F32, kind="ExternalInput")
    ln2_b = nc.dram_tensor("ln2_b", [D], F32, kind="ExternalInput")
    yT = nc.dram_tensor("yT", [D, NT], F32, kind="ExternalOutput")

    with tile.TileContext(nc) as tc, \
         nc.allow_low_precision(reason="bf16 matmul operands; fp32 spine"):
        for _ in range(reps):
            _emit(nc, tc, xT, x_own, w_q, w_k, w_v, w_out, w1, w2, b1, b2,
                  ln1_w, ln1_b, ln2_w, ln2_b, yT, phases=phases)
    nc.compile()
    return nc


def _emit(nc, tc, xT_d, xown_d, w_q, w_k, w_v, w_out, w1, w2, b1, b2,
          ln1_w, ln1_b, ln2_w, ln2_b, yT_d,
          phases=("qkv", "attn", "proj", "ffn")):
    # ---------------- whole-kernel pools ----------------
    with tc.tile_pool(name="const", bufs=1) as pc, \
         tc.tile_pool(name="pers", bufs=1) as pers, \
         tc.tile_pool(name="scratch", bufs=3) as sq_pool, \
         tc.tile_pool(name="vecs", bufs=4) as vec_pool, \
         tc.tile_pool(name="psacc", bufs=2, space="PSUM") as psacc, \
         tc.tile_pool(name="pspt", bufs=2, space="PSUM") as pspt, \
         tc.tile_pool(name="psout", bufs=2, space="PSUM") as psout:

        # ---------------- constants ----------------
        ones_f32 = pc.tile([P, 2 * P], F32)
        nc.vector.memset(ones_f32[:], 1.0)
        ones_col = pc.tile([P, 1], BF16)          # lhsT for partition-sums
        nc.vector.tensor_copy(ones_col[:], ones_f32[:, 0:1])
        ones_row = pc.tile([1, P], F32)           # lhsT for exact broadcasts
        nc.vector.tensor_copy(ones_row[:], ones_f32[0:1, 0:P])
        eps_sb = pc.tile([1, 1], F32)
        nc.vector.memset(eps_sb[:], EPS)
        b1_sb = pc.tile([P, FC], F32)
        nc.sync.dma_start(b1_sb[:], b1.ap().rearrange("(c p) -> p c", p=P))
        b2_sb = pc.tile([P, DM], F32)
        nc.sync.dma_start(b2_sb[:], b2.ap().rearrange("(c p) -> p c", p=P))
        lnw1_sb = pc.tile([P, DM], F32)
        nc.sync.dma_start(lnw1_sb[:], ln1_w.ap().rearrange("(c p) -> p c", p=P))
        lnb1_sb = pc.tile([P, DM], F32)
        nc.sync.dma_start(lnb1_sb[:], ln1_b.ap().rearrange("(c p) -> p c", p=P))
        lnw2_sb = pc.tile([P, DM], F32)
        nc.sync.dma_start(lnw2_sb[:], ln2_w.ap().rearrange("(c p) -> p c", p=P))
        lnb2_sb = pc.tile([P, DM], F32)
        nc.sync.dma_start(lnb2_sb[:], ln2_b.ap().rearrange("(c p) -> p c", p=P))

        # persistent activations (z2 reuses xow's slot via tag rotation)
        QT = pers.tile([P, DM, NT], BF16)
        outT = pers.tile([P, DM, NT], BF16)
        xow = pers.tile([P, DM, NT], F32, tag="tc")  # own-token x (residual 1)
        xln1 = pers.tile([P, DM, NT], BF16)     # LN1 out (ffn matmul operand)

        nc.sync.dma_start(xow[:], _rearr(xown_d.ap()))

        def ln_apply(z_tile, writes):
            """LayerNorm over features of z_tile [P, DM, NT] (fp32).
            writes(k, src_f32_ap) stores chunk k."""
            # bf16 shadow for the PE stat reductions (errors average out)
            s1 = psacc.tile([1, NT], F32, tag="acc")
            s2 = psacc.tile([1, NT], F32, tag="acc")
            for k in range(DM):
                zb = sq_pool.tile([P, NT], BF16, tag="sq")
                nc.vector.tensor_copy(zb[:], z_tile[:, k, :])
                nc.tensor.matmul(s1[:], ones_col[:], zb[:],
                                 start=(k == 0), stop=(k == DM - 1))
                sq = sq_pool.tile([P, NT], BF16, tag="sq")
                nc.vector.tensor_mul(sq[:], zb[:], zb[:])
                nc.tensor.matmul(s2[:], ones_col[:], sq[:],
                                 start=(k == 0), stop=(k == DM - 1))
            mu = vec_pool.tile([1, NT], F32, tag="v")
            nc.vector.tensor_scalar_mul(mu[:], s1[:], 1.0 / D)
            var = vec_pool.tile([1, NT], F32, tag="v")
            nc.vector.tensor_scalar_mul(var[:], s2[:], 1.0 / D)
            musq = vec_pool.tile([1, NT], F32, tag="v")
            nc.vector.tensor_mul(musq[:], mu[:], mu[:])
            nc.vector.tensor_sub(var[:], var[:], musq[:])
            nc.scalar.activation(var[:], var[:], AF.Sqrt, bias=eps_sb[:])
            rec = vec_pool.tile([1, NT], F32, tag="v")
            nc.vector.reciprocal(rec[:], var[:])
            murf = vec_pool.tile([1, NT], F32, tag="v")
            nc.vector.tensor_mul(murf[:], mu[:], rec[:])
            R = psacc.tile([P, NT], F32, tag="acc")
            nc.tensor.matmul(R[:], ones_row[:], rec[:], start=True, stop=True)
            MR = psacc.tile([P, NT], F32, tag="acc")
            nc.tensor.matmul(MR[:], ones_row[:], murf[:], start=True, stop=True)
            for k in range(DM):
                t = sq_pool.tile([P, NT], F32, tag="sq")
                nc.vector.tensor_mul(t[:], z_tile[:, k, :], R[:])
                nc.vector.tensor_sub(t[:], t[:], MR[:])
                writes(k, t)

        with tc.tile_pool(name="ktp", bufs=1) as kt_pool, \
             tc.tile_pool(name="vpp", bufs=1) as vp_pool, \
             tc.tile_pool(name="w1p", bufs=4) as w1_pool:

            with tc.tile_pool(name="xpool", bufs=1) as px:
                xT = px.tile([P, DM, NSEQ], BF16)
                xTs = _rearr(xT_d.ap())
                for k in range(DM):
                    nc.sync.dma_start(xT[:, k, :], xTs[:, k, :])

                # -------- projections: Q, V, K (dense PE block) ----------
                with tc.tile_pool(name="wq", bufs=2) as wq_pool:
                    for qf in range(DM):
                        wq = wq_pool.tile([P, DM, P], BF16)
                        nc.sync.dma_start(wq[:], w_q.ap()[qf])
                        acc = psacc.tile([P, NT], F32, tag="acc")
                        for k in range(DM):
                            nc.tensor.matmul(acc[:], wq[:, k, :],
                                             xT[:, k, 0:NT],
                                             start=(k == 0), stop=(k == DM - 1))
                        nc.vector.tensor_copy(QT[:, qf, :], acc[:])

                vp = vp_pool.tile([P, JC, H * 65], BF16, tag="vp")
                vp_h = vp.rearrange("p j (h e) -> p j h e", e=65)
                nc.vector.tensor_copy(
                    vp_h[:, :, :, 64:65],
                    ones_f32.rearrange("p (a b c) -> p a b c", b=H, c=1))
                with tc.tile_pool(name="wv", bufs=2) as wv_pool:
                    for dvc in range(2):  # 512 v-features = 8 heads at a time
                        wv = wv_pool.tile([P, DM, 512], BF16)
                        nc.sync.dma_start(wv[:], w_v.ap()[dvc])
                        for jc in range(JC):
                            acc = psacc.tile([P, 512], F32, tag="acc")
                            for k in range(DM):
                                nc.tensor.matmul(
                                    acc[:], xT[:, k, jc * P:(jc + 1) * P],
                                    wv[:, k, :],
                                    start=(k == 0), stop=(k == DM - 1))
                            nc.vector.tensor_copy(
                                vp_h[:, jc, dvc * 8:(dvc + 1) * 8, 0:64],
                                acc[:].rearrange("p (h e) -> p h e", e=64))

                kt = kt_pool.tile([P, DM, NSEQ], BF16, tag="kt")
                with tc.tile_pool(name="wk", bufs=2) as wk_pool:
                    for kf in range(DM):
                        wk = wk_pool.tile([P, DM, P], BF16)
                        nc.sync.dma_start(wk[:], w_k.ap()[kf])
                        for t in range(TC):
                            acc = psacc.tile([P, 512], F32, tag="acc")
                            for k in range(DM):
                                nc.tensor.matmul(
                                    acc[:], wk[:, k, :],
                                    xT[:, k, t * 512:(t + 1) * 512],
                                    start=(k == 0), stop=(k == DM - 1))
                            nc.vector.tensor_copy(
                                kt[:, kf, t * 512:(t + 1) * 512], acc[:])

            # -------- prefetch FFN1 weights during attention -------------
            w1ts = []
            if "ffn" in phases:
                for fg in range(DFF // 512):
                    w1t = w1_pool.tile([P, DM, 512], BF16, tag="w1",
                                       name=f"w1t{fg}")
                    nc.gpsimd.dma_start(w1t[:], w1.ap()[fg])
                    w1ts.append(w1t)

            # -------- attention (8 head-pairs) ---------------------------
            if "attn" not in phases:      # timing-bisect stub
                for k in range(DM):
                    nc.vector.tensor_copy(outT[:, k, :], QT[:, k, :])
            if "attn" in phases:
                with tc.tile_pool(name="pt", bufs=8) as pt_pool:
                    for hp in range(HPAIRS):
                        oacc = [psout.tile([65, NT], F32, tag="o",
                                           name=f"oacc{i}") for i in range(2)]
                        for jc in range(JC):
                            pt_ps = pspt.tile([P, 2 * NT], F32, tag="pt")
                            for i in range(2):
                                rows = slice(64 * i, 64 * i + 64)
                                nc.tensor.matmul(
                                    pt_ps[:, i * NT:(i + 1) * NT],
                                    kt[rows, hp, jc * P:(jc + 1) * P],
                                    QT[rows, hp, :],
                                    start=True, stop=True)
                            pt_sb = pt_pool.tile([P, 2 * NT], BF16, tag="ptsb")
                            nc.scalar.activation(pt_sb[:], pt_ps[:], AF.Exp,
                                                 scale=SCALE)
                            for i in range(2):
                                h = 2 * hp + i
                                nc.tensor.matmul(
                                    oacc[i][:],
                                    vp[:, jc, h * 65:(h + 1) * 65],
                                    pt_sb[:, i * NT:(i + 1) * NT],
                                    start=(jc == 0), stop=(jc == JC - 1))
                        for i in range(2):
                            rec = vec_pool.tile([1, NT], F32, tag="v")
                            nc.vector.reciprocal(rec[:], oacc[i][64:65, :])
                            bc = pspt.tile([64, NT], F32, tag="pt")
                            nc.tensor.matmul(bc[:], ones_row[:, 0:64], rec[:],
                                             start=True, stop=True)
                            bc_sb = sq_pool.tile([P, NT], F32, tag="sq")
                            nc.vector.tensor_copy(bc_sb[0:64, :], bc[:])
                            nc.vector.tensor_mul(
                                outT[64 * i:64 * i + 64, hp, :],
                                oacc[i][0:64, :], bc_sb[0:64, :])

            # -------- output projection + residual 1 ---------------------
            z1 = kt_pool.tile([P, DM, NT], F32, tag="kt")  # reuses kt slot
            with tc.tile_pool(name="wo", bufs=2) as wo_pool:
                for ef in range(DM):
                    wo = wo_pool.tile([P, DM, P], BF16)
                    nc.sync.dma_start(wo[:], w_out.ap()[ef])
                    acc = psacc.tile([P, NT], F32, tag="acc")
                    for k in range(DM):
                        nc.tensor.matmul(acc[:], wo[:, k, :], outT[:, k, :],
                                         start=(k == 0), stop=(k == DM - 1))
                    nc.vector.tensor_add(z1[:, ef, :], acc[:], xow[:, ef, :])

            # -------- LN1 ------------------------------------------------
            xln1f = vp_pool.tile([P, DM, NT], F32, tag="vp")  # reuses vp slot

            def write_xln1(k, t):
                nc.vector.tensor_scalar(xln1f[:, k, :], t[:],
                                        lnw1_sb[:, k:k + 1],
                                        lnb1_sb[:, k:k + 1],
                                        op0=mybir.AluOpType.mult,
                                        op1=mybir.AluOpType.add)
                nc.vector.tensor_copy(xln1[:, k, :], xln1f[:, k, :])
            ln_apply(z1, write_xln1)

            if "ffn" not in phases:   # timing-bisect stub: LN2 input
                z2 = pers.tile([P, DM, NT], F32, tag="tc")
                for k in range(DM):
                    nc.vector.tensor_copy(z2[:, k, :], z1[:, k, :])

            # -------- FFN ------------------------------------------------
            if "ffn" in phases:
                with tc.tile_pool(name="hpool", bufs=1) as ph:
                    hT = ph.tile([P, FC, NT], BF16)
                    for fg in range(DFF // 512):
                        w1t = w1ts[fg]
                        for f4 in range(4):
                            f = fg * 4 + f4
                            acc = psacc.tile([P, NT], F32, tag="acc")
                            for k in range(DM):
                                nc.tensor.matmul(
                                    acc[:], w1t[:, k, f4 * P:(f4 + 1) * P],
                                    xln1[:, k, :],
                                    start=(k == 0), stop=(k == DM - 1))
                            nc.scalar.activation(hT[:, f, :], acc[:], AF.Gelu,
                                                 bias=b1_sb[:, f:f + 1])

                    with tc.tile_pool(name="w2p", bufs=2) as w2_pool:
                        z2 = pers.tile([P, DM, NT], F32, tag="tc")  # xow slot
                        for ef in range(DM):
                            w2t = w2_pool.tile([P, FC, P], BF16)
                            nc.gpsimd.dma_start(w2t[:], w2.ap()[ef])
                            acc = psacc.tile([P, NT], F32, tag="acc")
                            for k in range(FC):
                                nc.tensor.matmul(acc[:], w2t[:, k, :],
                                                 hT[:, k, :],
                                                 start=(k == 0),
                                                 stop=(k == FC - 1))
                            t = sq_pool.tile([P, NT], F32, tag="sq")
                            nc.vector.tensor_scalar_add(t[:], acc[:],
                                                        b2_sb[:, ef:ef + 1])
                            nc.vector.tensor_add(z2[:, ef, :], t[:],
                                                 xln1f[:, ef, :])

        # -------- LN2 -> output ------------------------------------------
        with tc.tile_pool(name="outstage", bufs=2) as out_pool:
            yT_r = _rearr(yT_d.ap())

            def write_out(k, t):
                o = out_pool.tile([P, NT], F32)
                nc.vector.tensor_scalar(o[:], t[:],
                                        lnw2_sb[:, k:k + 1],
                                        lnb2_sb[:, k:k + 1],
                                        op0=mybir.AluOpType.mult,
                                        op1=mybir.AluOpType.add)
                nc.sync.dma_start(yT_r[:, k, :], o[:])
            ln_apply(z2, write_out)  # noqa: F821


def _get_nc():
    global _NC_CACHE
    if _NC_CACHE is None:
        _NC_CACHE = _build_nc()
    return _NC_CACHE


def _tile_w(W, out_cols):
    """[Din, Dout] f32 -> bf16 [Dout//out_cols, 128, Din//128, out_cols]
    so each output-chunk's weights are one contiguous DMA slab."""
    import ml_dtypes
    Din, Dout = W.shape
    t = W.astype(ml_dtypes.bfloat16).reshape(Din // P, P,
                                             Dout // out_cols, out_cols)
    return np.ascontiguousarray(t.transpose(2, 1, 0, 3))


def make_in_maps(x, w_qkv, w_out, ln1_w, ln1_b, w1, b1, w2, b2,
                 ln2_w, ln2_b):
    import ml_dtypes
    bf = ml_dtypes.bfloat16
    x = np.ascontiguousarray(np.asarray(x, dtype=np.float32))
    w_qkv = np.asarray(w_qkv, np.float32)
    shared = {
        "w_q": _tile_w(w_qkv[:, 0:D], P),
        "w_k": _tile_w(w_qkv[:, D:2 * D], P),
        "w_v": _tile_w(w_qkv[:, 2 * D:3 * D], 512),
        "w_out": _tile_w(np.asarray(w_out, np.float32), P),
        "w1": _tile_w(np.asarray(w1, np.float32), 512),
        "w2": _tile_w(np.asarray(w2, np.float32), P),
        "b1": np.asarray(b1, np.float32),
        "b2": np.asarray(b2, np.float32),
        "ln1_w": np.asarray(ln1_w, np.float32),
        "ln1_b": np.asarray(ln1_b, np.float32),
        "ln2_w": np.asarray(ln2_w, np.float32),
        "ln2_b": np.asarray(ln2_b, np.float32),
    }
    in_maps = []
    for c in range(8):
        b, q = divmod(c, 4)
        xT = np.ascontiguousarray(x[b].T)             # [D, NSEQ]
        # rotate so this core's own tokens are always columns [0, NT)
        xTr = np.ascontiguousarray(np.roll(xT, -q * NT, axis=1))
        in_maps.append({
            "xT": np.ascontiguousarray(xTr.astype(bf)),
            "x_own": np.ascontiguousarray(xTr[:, 0:NT]),
            **shared,
        })
    return in_maps


def kernel(x, w_qkv, w_out, ln1_w, ln1_b, w1, b1, w2, b2, ln2_w, ln2_b):
    in_maps = make_in_maps(x, w_qkv, w_out, ln1_w, ln1_b, w1, b1, w2, b2,
                           ln2_w, ln2_b)
    nc = _get_nc()
    res = run_bass_kernel_spmd(nc, in_maps, list(range(8)))

    out = np.empty((B, NSEQ, D), np.float32)
    for c in range(8):
        b, q = divmod(c, 4)
        out[b, q * NT:(q + 1) * NT, :] = res.results[c]["yT"].T
    return out
